# revision 12
# baseline (speedup 1.0000x reference)
"""JointCCSA loss kernel for 8 Trainium2 NeuronCores — v2 (fp8 DoubleRow).

reference:
    dists = cdist(X, X)                                  (bs, bs)
    sa_loss = 0.5 * sum[ same_y & ds_lt ] dists / n_sa
    s_loss  = 0.5 * sum[ y_lt  & ds_lt ] relu(1 - dists) / n_s

Design (data-parallel rows, 512 rows/core, 8 col-panels of 512):
  * X rounded to fp8 e4m3 host-side; lhs = -2*x (exact in fp8).  Distances
    are the exact distances of the rounded points: d2 = psum(-2x.x) + sq_j
    + sq_i with sq computed from the rounded values in f64 host-side.
  * Per (row-chunk rc, panel p) tile [128 x 512]:
      - 2 fp8 DoubleRow matmuls (K=2x128 each) accumulate -2*X_loc @ X^T
      - 1 bf16 K=2 matmul adds sq_j (hi+lo rows) into the same PSUM bank
      - ACT: dist = Sqrt(psum + (sq_i + C0)) -> bf16 SBUF
      - DVE: ds8 = fp8(dist - 32)   (shift centers fp8 precision: ulp
        0.25 instead of 2 at dist~32)                      [dq slot 0]
      - DVE: q = min(dist - 1, 0) = -relu(1 - dist) fp8    [dq slot 1]
      - 1 fp8 DoubleRow stats matmul: T[0:12]  += U_sa^T @ ds8
                                      T[12:24] += U_s^T  @ q
        accumulated over the panel's 4 row-chunks in PSUM.
    The stats matmul is delayed 3 tiles so PE never stalls on ACT/DVE.
  * T [24, 512] per panel DMA'd PSUM->DRAM directly.
  * Host: sa_sum = sum_j T[combo_j, j] + 32*N_sa; s_hinge = -sum_j
    T[12+combo_j, j]  (mask(i,j) = sum_r U(i,r)[combo_j==r]; diagonal
    auto-excluded since U_sa(i, combo_i) = U_s(i, combo_i) = 0).
"""

import numpy as np
import ml_dtypes
from contextlib import ExitStack

import concourse.bass as bass
import concourse.tile as tile
from concourse import mybir
from concourse.vector_clock import ScopedClock
from concourse.bass_utils import run_bass_kernel_spmd

BS = 4096
D = 512
NCORES = 8
MLOC = BS // NCORES          # 512 rows per core
MCH = MLOC // 128            # 4 row chunks per core
NP = 8                       # col panels
PW = 512                     # panel width
C0 = 0.0625                  # sqrt-safety bias added into sq_i
SHIFT = 32.0                 # dist recentering for fp8 stats precision
F8 = ml_dtypes.float8_e4m3
BF16 = ml_dtypes.bfloat16


# ---------------------------------------------------------------------------
# Patch: this walrus build allows only ONE sync-wait on a CTRL-type (Drain)
# instruction; Tile's final drain aggregates many.  Spread them over
# single-wait SP nops.
def _patched_drain_and_barrier(self, tick_clock, wait_clock):
    nc = self.nc
    coll = nc.sync.nop(nofuse=True, hint="drain_wait_collector")
    wait_clock.add_sem_waits(coll.ins, ScopedClock({None: tick_clock.global_clock}))
    si = coll.ins.sync_info
    waits = list(si.on_wait) if si is not None else []
    if len(waits) > 1:
        si.on_wait = [waits[0]]
        for w in waits[1:]:
            n = nc.sync.nop(nofuse=True, hint="drain_wait_extra")
            n.ins.sync_info = mybir.SyncInfo(on_wait=[w], on_update=[])
    nc.sync.drain()
    nc.all_engine_barrier()
    assert self.sems is not None
    popped = nc._tile_sem_poison_stack.pop()
    assert popped is self._sem_poison
    nc.clear_and_free_semaphores(list(self.sems.allocated().values()))
    nc.all_engine_barrier()


tile.TileContext._drain_and_barrier = _patched_drain_and_barrier


def _split_waits(nc, maxw=1):
    """Hoist extra sync-waits from every instruction onto same-engine NoOps
    (this walrus build rejects instructions with more than ~1 wait)."""
    for fn in nc.m.functions:
        for blk in fn.blocks:
            newlist = []
            for inst in blk.instructions:
                si = getattr(inst, "sync_info", None)
                if si is not None and len(si.on_wait) > maxw:
                    waits = list(si.on_wait)
                    for i, w in enumerate(waits[maxw:]):
                        nop = mybir.InstNoOp(
                            name=f"{inst.name}-wsplit{i}",
                            sync_info=mybir.SyncInfo(on_wait=[w], on_update=[]),
                            bass_nofuse=True,
                            engine=inst.engine,
                        )
                        nc.register_instruction(nop)
                        newlist.append(nop)
                    si.on_wait = waits[:maxw]
                newlist.append(inst)
            blk.instructions[:] = newlist
# ---------------------------------------------------------------------------

_NC_CACHE = {}


def build_program():
    if "nc" in _NC_CACHE:
        return _NC_CACHE["nc"]
    f32 = mybir.dt.float32
    bf16 = mybir.dt.bfloat16
    f8 = mybir.dt.float8e4
    DR = mybir.MatmulPerfMode.DoubleRow

    nc = bass.Bass()
    lhs_d = nc.declare_dram_parameter("lhs", [128, MCH, 2, 2, 128], f8, isOutput=False)
    rhs_d = nc.declare_dram_parameter("rhs", [128, 2, 2, BS], f8, isOutput=False)
    sqhl_d = nc.declare_dram_parameter("sqhl", [2, BS], bf16, isOutput=False)
    sqb_d = nc.declare_dram_parameter("sqb", [128, MCH], f32, isOutput=False)
    uu_d = nc.declare_dram_parameter("uu", [128, MCH, 2, 32], f8, isOutput=False)
    one_d = nc.declare_dram_parameter("one", [2, 128], bf16, isOutput=False)
    out_d = nc.declare_dram_parameter("out", [32, BS], f32, isOutput=True)

    with tile.TileContext(nc) as tc, ExitStack() as ctx:
        singles = ctx.enter_context(tc.tile_pool(name="singles", bufs=1))
        work = ctx.enter_context(tc.tile_pool(name="work", bufs=4))
        pd2 = ctx.enter_context(tc.tile_pool(name="pd2", bufs=3, space="PSUM"))
        pT = ctx.enter_context(tc.tile_pool(name="pT", bufs=2, space="PSUM"))

        # --- input DMAs, spread across the SP and ACT queues in need-order.
        BX = singles.tile([128, 2, 2, BS], f8)
        nc.sync.dma_start(out=BX[:, :, :, 0:PW], in_=rhs_d[:, :, :, 0:PW])
        ONE = singles.tile([2, 128], bf16)
        nc.sync.dma_start(out=ONE, in_=one_d[:, :])
        SQ = singles.tile([2, BS], bf16)
        nc.sync.dma_start(out=SQ, in_=sqhl_d[:, :])
        SQB = singles.tile([128, MCH], f32)
        nc.sync.dma_start(out=SQB, in_=sqb_d[:, :])
        AX = singles.tile([128, MCH, 2, 2, 128], f8)
        nc.scalar.dma_start(out=AX, in_=lhs_d[:, :, :, :, :])
        UU = singles.tile([128, MCH, 2, 32], f8)
        nc.scalar.dma_start(out=UU, in_=uu_d[:, :, :, :])
        nc.scalar.dma_start(out=BX[:, :, :, PW:2304], in_=rhs_d[:, :, :, PW:2304])
        nc.sync.dma_start(out=BX[:, :, :, 2304:BS], in_=rhs_d[:, :, :, 2304:BS])

        # Preload the sqrt activation table during the DMA preamble.
        warm = singles.tile([1, 2], f32)
        nc.vector.memset(warm, 1.0)
        nc.scalar.activation(out=warm, in_=warm,
                             func=mybir.ActivationFunctionType.Sqrt)

        NT = NP * MCH              # 32 tiles, t -> (p = t//MCH, rc = t%MCH)
        DELAY = 3                  # stats matmul lags 3 tiles
        Ttiles = {}
        dq_tiles = {}

        def dist_stage(t):
            p, rc = divmod(t, MCH)
            jsl = slice(p * PW, (p + 1) * PW)
            d2 = pd2.tile([128, PW], mybir.dt.float32, name="d2")
            nc.tensor.matmul(d2, AX[:, rc, 0], BX[:, 0, :, jsl],
                             start=True, stop=False, perf_mode=DR)
            nc.tensor.matmul(d2, AX[:, rc, 1], BX[:, 1, :, jsl],
                             start=False, stop=False, perf_mode=DR)
            nc.tensor.matmul(d2, ONE, SQ[:, jsl], start=False, stop=True)
            db = work.tile([128, PW], mybir.dt.bfloat16, name="db")
            nc.scalar.activation(out=db, in_=d2,
                                 func=mybir.ActivationFunctionType.Sqrt,
                                 bias=SQB[:, rc:rc + 1], scale=1.0)
            dq = work.tile([128, 2, PW], mybir.dt.float8e4, name="dq")
            nc.vector.tensor_scalar(out=dq[:, 0, :], in0=db,
                                    scalar1=float(SHIFT), scalar2=None,
                                    op0=mybir.AluOpType.subtract)
            nc.vector.tensor_scalar(out=dq[:, 1, :], in0=db,
                                    scalar1=1.0, scalar2=0.0,
                                    op0=mybir.AluOpType.subtract,
                                    op1=mybir.AluOpType.min)
            dq_tiles[t] = dq

        def stats_stage(t):
            p, rc = divmod(t, MCH)
            if rc == 0:
                Ttiles[p] = pT.tile([32, PW], mybir.dt.float32, name="T")
            nc.tensor.matmul(Ttiles[p], UU[:, rc], dq_tiles.pop(t),
                             start=(rc == 0), stop=(rc == MCH - 1),
                             perf_mode=DR)
            if rc == MCH - 1:
                jsl = slice(p * PW, (p + 1) * PW)
                Tsb = work.tile([32, PW], mybir.dt.float32, name="Tsb")
                nc.vector.tensor_copy(out=Tsb, in_=Ttiles.pop(p))
                nc.sync.dma_start(out=out_d[:, jsl], in_=Tsb)

        for t in range(NT):
            dist_stage(t)
            if t >= DELAY:
                stats_stage(t - DELAY)
        for t in range(NT - DELAY, NT):
            stats_stage(t)

    _split_waits(nc)
    _NC_CACHE["nc"] = nc
    return nc


def prepare_inputs(X, ds, y):
    X = np.asarray(X, dtype=np.float32)
    ds = np.asarray(ds).astype(np.int64)
    y = np.asarray(y).astype(np.int64)

    X8 = X.astype(F8)
    Xd = X8.astype(np.float64)
    sq = (Xd * Xd).sum(axis=1)
    sq32 = sq.astype(np.float32)
    sq_hi = sq32.astype(BF16)
    sq_lo = (sq32 - sq_hi.astype(np.float32)).astype(BF16)
    sqhl = np.ascontiguousarray(np.stack([sq_hi, sq_lo], axis=0))   # (2, BS)

    L8 = (-2.0 * Xd).astype(F8)

    # rhs[c, dr, kt, j] = X8[j, 128*(2dr+kt)+c]
    rhs = np.ascontiguousarray(
        X8.T.reshape(2, 2, 128, BS).transpose(2, 0, 1, 3))          # (128,2,2,BS)

    # masks, rank-12:  r = c*3 + a
    cc = (np.arange(12) // 3)[None, :]
    aa = (np.arange(12) % 3)[None, :]
    U_sa = ((y[:, None] == cc) & (ds[:, None] < aa))
    U_s = ((y[:, None] < cc) & (ds[:, None] < aa))

    one = np.ones((2, 128), dtype=BF16)

    in_maps = []
    for c in range(NCORES):
        r0 = c * MLOC
        # lhs[c, rc, dr, kt, m] = L8[r0+128rc+m, 128*(2dr+kt)+c]
        lhs = np.ascontiguousarray(
            L8[r0:r0 + MLOC].T.reshape(2, 2, 128, MCH, 128)
            .transpose(2, 3, 0, 1, 4))                              # (128,MCH,2,2,128)
        sqb = np.ascontiguousarray(
            (sq32[r0:r0 + MLOC] + np.float32(C0)).reshape(MCH, 128).T)  # (128,MCH)
        # uu[i, rc, kt, r]: kt0 -> [U_sa | 0], kt1 -> [0 | U_s]
        uu = np.zeros((128, MCH, 2, 32), dtype=F8)
        Usa_c = U_sa[r0:r0 + MLOC].reshape(MCH, 128, 12)
        Us_c = U_s[r0:r0 + MLOC].reshape(MCH, 128, 12)
        uu[:, :, 0, 0:12] = Usa_c.transpose(1, 0, 2).astype(F8)
        uu[:, :, 1, 12:24] = Us_c.transpose(1, 0, 2).astype(F8)
        in_maps.append({
            "lhs": lhs,
            "rhs": rhs,
            "sqhl": sqhl,
            "sqb": sqb.astype(np.float32),
            "uu": uu,
            "one": one,
        })
    return in_maps


def finish(results, ds, y, n_classes, n_domains):
    ds = np.asarray(ds).astype(np.int64)
    y = np.asarray(y).astype(np.int64)
    n_classes = int(n_classes)
    n_domains = int(n_domains)
    combo = (y * 3 + ds).astype(np.int64)
    jj = np.arange(BS)

    # exact count of masked sa-pairs covered per core (for the fp8 shift)
    cc = (np.arange(12) // 3)[None, :]
    aa = (np.arange(12) % 3)[None, :]
    U_sa = ((y[:, None] == cc) & (ds[:, None] < aa)).astype(np.int64)
    nglob = np.bincount(combo, minlength=12).astype(np.int64)

    sa_sum = 0.0
    s_hinge = 0.0
    for c in range(NCORES):
        T = np.asarray(results[c]["out"], dtype=np.float64)   # (24, BS)
        cnt_c = U_sa[c * MLOC:(c + 1) * MLOC].sum(axis=0)
        sa_sum += T[0:12][combo, jj].sum() + SHIFT * float(cnt_c @ nglob)
        s_hinge -= T[12:24][combo, jj].sum()

    n_sa = n_classes * (n_domains * (n_domains - 1) // 2)
    n_s = (n_classes * (n_classes - 1) // 2) * (n_domains * (n_domains - 1) // 2)
    sa_loss = 0.5 * sa_sum / n_sa
    s_loss = 0.5 * s_hinge / n_s
    return np.array([sa_loss, s_loss], dtype=np.float32)


def run_device(in_maps, trace=False, **kw):
    nc = build_program()
    return run_bass_kernel_spmd(nc, in_maps, core_ids=list(range(NCORES)),
                                trace=trace, **kw)


def kernel(X, ds, y, n_classes, n_domains):
    in_maps = prepare_inputs(X, ds, y)
    res = run_device(in_maps)
    return finish(res.results, ds, y, n_classes, n_domains)
